# revision 87
# baseline (speedup 1.0000x reference)
"""BasicTransformerBlock on 8 TRN2 NeuronCores.

Strategy: pure data parallelism over the batch (B=8, one batch element per
core). Inside each core the block runs with feature-major activations:
  - dense projections: bf16 weights (lhsT) x bf16 activations (rhs)
  - the residual stream is kept in bf16 end-to-end (except the final
    LN3 stage, which stays f32 for the f32 output DMA); rel-err budget
    is 2e-2 and bf16 residual sims at ~8.6e-3
  - attention score/AV in bf16; softmax denominator via a ones-column
    appended to V (extra PSUM partition row), broadcast back across
    partitions with a K=1 ones-matmul on the PE (no DRAM roundtrip)
  - layernorm stats via ones-vector matmuls; mean/rstd rows broadcast
    across partitions with K=1 ones-matmuls
  - attention outputs normalized directly into (head*96+dh)-packed
    128-row tiles (for BOTH attentions) so the output projections
    contract K=128 chunks of the natural wo layout
  - GEGLU FF: ff_w1 streamed per-128-col chunk in bf16, FF2 partial
    products accumulated into 5 dedicated PSUM banks interleaved with
    FF1, so the PE stream stays dense (no mfull staging buffer)
The host pre-transposes x/context, packs all weights into per-partition
contiguous [128, ...] layouts in bf16 (sa q/k head-major so the first
head's slab lands fast at startup), reshapes biases per-partition, and
transposes the output back.
"""

import math

import numpy as np
import ml_dtypes

import concourse.bass as bass
import concourse.mybir as mybir
import concourse.tile as tile
from concourse import bacc
from concourse.bass_utils import run_bass_kernel_spmd

F32 = mybir.dt.float32
F32R = mybir.dt.float32r
BF16 = mybir.dt.bfloat16
AF = mybir.ActivationFunctionType
OP = mybir.AluOpType

P = 128
B = 8
NT = 1024          # query tokens
D = 640            # model dim; 5 chunks of 128
KC = 5
NH = 8             # heads
DH = 80            # head dim
CM = 77            # context tokens
CD = 768           # context dim; 6 chunks
CKC = 6
FH = 2560          # GEGLU half hidden; 20 chunks of 128
FJ = 20
NC = 2             # token chunks of 512
NW = 512
ISCALE = 1.0 / math.sqrt(DH)
LN_EPS = 1e-5

# packed-o layout: head h at rows 96h..96h+80 (96-pitch padding keeps
# every slice's partition base 32-aligned; the zero-padded wo rows make
# the pad partitions contribute nothing to the out-projection).
OPITCH = 96
OKC = NH * OPITCH // P  # 6 chunks of 128 rows
# each head writes its full 96-row pitch: rows 80..96 of po are exact
# zeros (memset pad columns of v_aug), so the pad partitions get zeroed
# as part of the normalize write.  Engines only allow partition-offset
# accesses of <=32 rows (quadrant windows), so any piece whose source or
# destination base is nonzero is cut into 32-row pieces.
O_SPLITS = []
for _h in range(NH):
    _r0, _r1 = _h * OPITCH, (_h + 1) * OPITCH
    _sl = []
    _r = _r0
    while _r < _r1:
        _c = _r // P
        _l = min(_r1, (_c + 1) * P) - _r
        _poff, _rs = _r % P, _r - _r0
        if _poff == 0 and _rs == 0:
            _sl.append((_c, _poff, _rs, _l))
        else:
            for _o in range(0, _l, 32):
                _sl.append((_c, _poff + _o, _rs + _o, min(32, _l - _o)))
        _r += _l
    O_SPLITS.append(_sl)

# CA keeps the simpler 128-pitch o layout: one normalize write per head
# (DVE op cost is column-count driven, so fewer/wider writes beat the
# denser 96-pitch packing there), at the price of 8 instead of 6
# K-chunks in the CA out-projection.
SPLITS128 = [[(_h, 0, 0, OPITCH)] for _h in range(NH)]


def _emit(nc, tc, apply_gb):
    d = nc._kd  # dram handles dict
    with (
        tc.tile_pool(name="sb", bufs=1) as sb,
        tc.tile_pool(name="ps", bufs=1, space="PSUM") as ps,
    ):
        _emit_body(nc, tc, d, sb, ps, apply_gb)


def _emit_body(nc, tc, d, sb, ps, apply_gb):
    def bank(name):
        return ps.tile([P, NW], F32, tag="bank", bufs=8, name=name)

    # ---------------- critical-path loads first: head-0 wq/wk + xTb ----
    # sa q/k weights are head-major [NH, P, KC, DH] in DRAM so head 0's
    # slab (102KB) gates the first matmul instead of the full 819KB.
    w_sa_q = sb.tile([P, NH, KC, DH], BF16, tag="w_sa_q", name="w_sa_q")
    w_sa_k = sb.tile([P, NH, KC, DH], BF16, tag="w_sa_k", name="w_sa_k")
    nc.sync.dma_start(w_sa_q[:, 0], d["sa_wq_p"][0])
    nc.scalar.dma_start(w_sa_k[:, 0], d["sa_wk_p"][0])
    # token-chunk 0 of xTb on the sync queue, chunk 1 on the ACT queue, so
    # the first Q/K projections of both chunks are fed in parallel; V
    # weights go down the gpsimd (SWDGE) queue which is otherwise idle.
    # xTb is the startup DMA bottleneck (~1.3MB, single HWDGE queue runs
    # ~122GB/s): chunks c0-c2 of each token half go down sync/scalar, the
    # c3/c4 chunks down the faster gpsimd SWDGE queue, ahead of wv.
    xTb = sb.tile([P, KC, NT], BF16, tag="xb", bufs=1, name="xTb")
    xsrc = d["xT_bf"].rearrange("(c p) n -> p c n", p=P)
    for ncq in range(NC):
        ncs = slice(ncq * NW, (ncq + 1) * NW)
        eng = nc.sync if ncq == 0 else nc.scalar
        for c in range(3):
            eng.dma_start(xTb[:, c, ncs], xsrc[:, c, ncs])
    for ncq in range(NC):
        ncs = slice(ncq * NW, (ncq + 1) * NW)
        for c in range(3, KC):
            nc.gpsimd.dma_start(xTb[:, c, ncs], xsrc[:, c, ncs])
    w_sa_v = sb.tile([P, KC, D], BF16, tag="w_sa_v", name="w_sa_v")
    nc.gpsimd.dma_start(w_sa_v, d["sa_wv_p"][:, :, :])
    nc.sync.dma_start(w_sa_q[:, 1:],
                      d["sa_wq_p"][1:].rearrange("h p c e -> p h c e"))
    nc.scalar.dma_start(w_sa_k[:, 1:],
                        d["sa_wk_p"][1:].rearrange("h p c e -> p h c e"))

    def w_load(key, shape, name, engine=None):
        t = sb.tile(shape, BF16, tag=name, bufs=1, name=name)
        (engine or nc.gpsimd).dma_start(t, d[key][:, :, :])
        return t

    # ---------------- constant / bias tiles ----------------
    ctxT = sb.tile([P, CKC, CM], BF16, tag="ctxT", bufs=1, name="ctxT")
    nc.gpsimd.dma_start(ctxT, d["ctxT_bf"].rearrange("(c p) m -> p c m", p=P))
    w_ca_k = w_load("ca_wk_p", [P, CKC, D], "w_ca_k")
    w_ca_v = w_load("ca_wv_p", [P, CKC, D], "w_ca_v")
    # all [P, x] fp32 constants in one packed DMA (SWDGE fixed cost ~2us
    # per dma_start; seven tiny loads would stall the early queue)
    NBC = 3 * KC + 2 * FJ + 1
    bias_t = sb.tile([P, NBC], F32, tag="bias", name="bias_t")
    nc.gpsimd.dma_start(bias_t, d["bias_p"][:, :])
    b_sa_bo = bias_t[:, 0:KC]
    b_ca_bo = bias_t[:, KC:2 * KC]
    b_ff2 = bias_t[:, 2 * KC:3 * KC]
    b_f1a = bias_t[:, 3 * KC:3 * KC + FJ]
    b_f1g = bias_t[:, 3 * KC + FJ:3 * KC + 2 * FJ]
    epst = bias_t[:, NBC - 1:NBC]
    ones_r = sb.tile([P, 1], F32R, tag="ones", name="ones_r")
    nc.gpsimd.dma_start(ones_r, d["ones"][:, :])
    ones_b = sb.tile([P, 1], BF16, tag="ones_b", name="ones_b")
    nc.gpsimd.dma_start(ones_b, d["ones_b"][:, :])
    onesrow = sb.tile([1, P], F32R, tag="onesrow", name="onesrow")
    nc.gpsimd.dma_start(onesrow, d["onesrow"][:, :])

    def act_table_hint(func, tag, src=None):
        """tiny dummy activation: forces the ACT table load for `func` to
        happen here (ACT idle) instead of on the critical path."""
        t = sb.tile([1, 8], F32, tag="dummy", bufs=2, name=f"dm_{tag}")
        nc.scalar.activation(t, bias_t[0:1, 0:8] if src is None else src,
                             func)
    lngb = {}
    if apply_gb:
        for ln in (1, 2, 3):
            for gb in ("g", "b"):
                t = sb.tile([P, KC], F32, tag=f"ln{ln}{gb}", name=f"ln{ln}{gb}")
                nc.gpsimd.dma_start(t, d[f"ln{ln}_{gb}_p"][:, :])
                lngb[(ln, gb)] = t

    w_sa_o = w_load("sa_wo_p", [P, OKC, D], "w_sa_o")
    w_ca_q = w_load("ca_wq_p", [P, KC, D], "w_ca_q")
    w_ca_o = w_load("ca_wo_p", [P, NH, D], "w_ca_o")

    def resid_tile(name, dt=BF16, tag="residb", bufs=2):
        return sb.tile([P, KC, NT], dt, tag=tag, bufs=bufs, name=name)

    def qk_proj(h, w_q, w_k, src, pref):
        qt = sb.tile([DH, NT], BF16, tag="qk", bufs=2, name=f"qt{pref}_{h}")
        kt = sb.tile([DH, NT], BF16, tag="qk", bufs=2, name=f"kt{pref}_{h}")
        for ncq in range(NC):
            ncs = slice(ncq * NW, (ncq + 1) * NW)
            pq = ps.tile([DH, NW], F32, tag="bank", bufs=8,
                         name=f"pq{pref}_{h}_{ncq}")
            pk = ps.tile([DH, NW], F32, tag="bank", bufs=8,
                         name=f"pk{pref}_{h}_{ncq}")
            for c in range(KC):
                nc.tensor.matmul(pq, w_q[:, h, c, :], src[:, c, ncs],
                                 start=(c == 0), stop=(c == KC - 1))
            for c in range(KC):
                nc.tensor.matmul(pk, w_k[:, h, c, :], src[:, c, ncs],
                                 start=(c == 0), stop=(c == KC - 1))
            nc.vector.tensor_copy(qt[:, ncs], pq)
            nc.vector.tensor_copy(kt[:, ncs], pk)
        return qt, kt

    # head-0 Q/K first: fills the PE while V weights land
    qk0 = qk_proj(0, w_sa_q, w_sa_k, xTb, "s")

    # ---------------- SA: V projection into V_aug ----------------
    v_aug = sb.tile([P, NH, NH, 97], BF16, tag="vaug", bufs=1, name="v_aug")
    nc.vector.memset(v_aug[:, :, :, 80:96], 0.0)
    nc.vector.memset(v_aug[:, :, :, 96:97], 1.0)
    for tch in range(NH):
        for g in range(2):  # two groups of 4 head-columns (320 wide)
            pv = ps.tile([P, 320], F32, tag="bank", bufs=8, name=f"pv_{tch}_{g}")
            for c in range(KC):
                nc.tensor.matmul(
                    pv, xTb[:, c, tch * P:(tch + 1) * P],
                    w_sa_v[:, c, g * 320:(g + 1) * 320],
                    start=(c == 0), stop=(c == KC - 1))
            nc.vector.tensor_copy(
                v_aug[:, tch, 4 * g:4 * g + 4, 0:80],
                pv.rearrange("p (s e) -> p s e", e=80))

    # preload the Exp table while ACT is still idle (sourced from xTb,
    # which lands ~10us before bias_t on the loaded gpsimd queue)
    act_table_hint(AF.Exp, "sa", src=xTb[0:1, 0, 0:8])

    # ---------------- CA: K/V projections (only need ctx) ----------------
    # emitted from inside the SA loop (h==1) so the PE doesn't wait on the
    # ca weight DMAs at kernel start
    kt_ca = sb.tile([DH, NH, CM], BF16, tag="ktca", bufs=1, name="kt_ca")
    vca_aug = sb.tile([CM, NH, 97], BF16, tag="vca", bufs=1, name="vca_aug")

    def emit_ca_kv():
        for h in range(NH):
            hs = slice(h * DH, (h + 1) * DH)
            pk = ps.tile([DH, CM], F32, tag="bank", bufs=8, name=f"pkca_{h}")
            for c in range(CKC):
                nc.tensor.matmul(pk, w_ca_k[:, c, hs], ctxT[:, c, :],
                                 start=(c == 0), stop=(c == CKC - 1))
            nc.vector.tensor_copy(kt_ca[:, h, :], pk)
        nc.vector.memset(vca_aug[:, :, 80:96], 0.0)
        nc.vector.memset(vca_aug[:, :, 96:97], 1.0)
        for g in range(2):
            pv = ps.tile([CM, 320], F32, tag="bank", bufs=8, name=f"pvca_{g}")
            for c in range(CKC):
                nc.tensor.matmul(pv, ctxT[:, c, :],
                                 w_ca_v[:, c, g * 320:(g + 1) * 320],
                                 start=(c == 0), stop=(c == CKC - 1))
            nc.vector.tensor_copy(vca_aug[:, 4 * g:4 * g + 4, 0:80],
                                  pv.rearrange("p (s e) -> p s e", e=80))

    AVLAG = 2

    def attn_scores_av(h, qt, kt, vaug_sl, mchunks, mpart, pref,
                       zrow_eng=None):
        """scores -> exp -> AV (lagged) for one head; the last AVLAG AV
        matmuls are deferred to the returned closure so the caller can
        slot independent PE work (next head's projections) in front of
        them while the exps complete.  zrow_eng picks the engine for the
        denominator-row copy: DVE in the ACT-bound SA phase, ACT in the
        DVE-leaning CA phase."""
        pos, tails = [], []
        for ncq in range(NC):
            ncs = slice(ncq * NW, (ncq + 1) * NW)
            po = ps.tile([97, NW], F32, tag="bank", bufs=8,
                         name=f"po{pref}_{h}_{ncq}")
            ets = []
            for mc in range(mchunks):
                pscore = ps.tile([mpart, NW], F32, tag="bank", bufs=8,
                                 name=f"psc{pref}_{h}_{ncq}_{mc}")
                nc.tensor.matmul(pscore, kt(mc), qt[:, ncs],
                                 start=True, stop=True)
                et = sb.tile([mpart, NW], BF16, tag="epool", bufs=4,
                             name=f"e{pref}_{h}_{ncq}_{mc}")
                nc.scalar.activation(et, pscore, AF.Exp, scale=ISCALE)
                ets.append(et)
                if mc >= AVLAG:  # AV lags its exp by AVLAG scores
                    nc.tensor.matmul(po, vaug_sl(mc - AVLAG), ets[mc - AVLAG],
                                     start=(mc == AVLAG), stop=False,
                                     skip_group_check=True)
            pos.append(po)
            tails.append(ets)

        zrows = []

        def finish():
            for ncq in range(NC):
                po, ets = pos[ncq], tails[ncq]
                for mc in range(max(0, mchunks - AVLAG), mchunks):
                    nc.tensor.matmul(po, vaug_sl(mc), ets[mc],
                                     start=(mc == 0),
                                     stop=(mc == mchunks - 1),
                                     skip_group_check=True)
                zrow = sb.tile([1, NW], F32R, tag="zrow", bufs=1,
                               name=f"zr{pref}_{h}_{ncq}")
                if zrow_eng is nc.vector:
                    nc.vector.tensor_copy(zrow, po[96:97, :])
                else:
                    nc.scalar.activation(zrow, po[96:97, :], AF.Copy)
                zrows.append(zrow)
        return pos, zrows, finish

    def attn_normalize(h, pos, zrows, o_tile, pref, splits):
        """softmax denominator -> PE broadcast -> recip -> packed write."""
        for ncq in range(NC):
            ncs = slice(ncq * NW, (ncq + 1) * NW)
            po = pos[ncq]
            pzb = ps.tile([OPITCH, NW], F32, tag="bank", bufs=8,
                          name=f"pzb{pref}_{h}_{ncq}")
            nc.tensor.matmul(pzb, onesrow[0:1, 0:OPITCH], zrows[ncq],
                             start=True, stop=True)
            zb = sb.tile([OPITCH, NW], F32, tag="zb", bufs=1,
                         name=f"zb{pref}_{h}_{ncq}")
            nc.vector.reciprocal_approx_fast(zb, pzb)
            for (c, poff, rs, ln) in splits[h]:
                nc.vector.tensor_tensor(
                    o_tile[poff:poff + ln, c, ncs],
                    po[rs:rs + ln, :], zb[rs:rs + ln, :], OP.mult)

    # ---------------- SA: per-head QK + attention ----------------
    # next head's Q/K projections are emitted between this head's AV and
    # its normalize so the PE never idles on the denominator chain.
    o_sa = sb.tile([P, OKC, NT], BF16, tag="opool", bufs=1, name="o_sa")
    # CA-style stagger: normalize(h-1) is emitted AFTER head h's scores,
    # so its pzb broadcast never waits on zrow with an empty PE queue
    qkh = qk0
    prev = None
    for h in range(NH):
        qt, kt = qkh
        pos, zrows, finish = attn_scores_av(
            h, qt,
            kt=lambda mc, _kt=kt: _kt[:, mc * P:(mc + 1) * P],
            vaug_sl=lambda mc, _h=h: v_aug[:, mc, _h, :],
            mchunks=NH, mpart=P, pref="s", zrow_eng=nc.vector)
        if prev is not None:
            attn_normalize(h - 1, prev[0], prev[1], o_sa, "s", O_SPLITS)
        if h + 1 < NH:
            qkh = qk_proj(h + 1, w_sa_q, w_sa_k, xTb, "s")
        if h == 1:
            emit_ca_kv()
        finish()
        prev = (pos, zrows)
    attn_normalize(NH - 1, prev[0], prev[1], o_sa, "s", O_SPLITS)

    def _ln_stats(res_out, psum_s, psum_q, ln_idx, ncq, ncs, do, is_f32):
        """sum and sum-of-squares matmuls for one 128-feature chunk."""
        src = res_out[:, do, ncs]
        sq = sb.tile([P, NW], F32R, tag="sq", bufs=1,
                     name=f"sq_{ln_idx}_{ncq}_{do}")
        nc.scalar.activation(sq, src.bitcast(F32) if is_f32 else src,
                             AF.Square)
        nc.tensor.matmul(psum_s, ones_r if is_f32 else ones_b, src,
                         start=(do == 0), stop=(do == KC - 1))
        nc.tensor.matmul(psum_q, ones_r, sq,
                         start=(do == 0), stop=(do == KC - 1))

    def out_proj_ln(ncq, wo_t, o_tile, bo_t, res_in, res_out, ln_idx, pref,
                    s_row, q_row, okc=OKC):
        """packed out-projection + residual + LN stats for one 512-token
        chunk; stats matmuls pipelined one chunk behind the residual."""
        ncs = slice(ncq * NW, (ncq + 1) * NW)
        if ncq == 0:
            # squares also exist in the rsqrt table; switching now keeps
            # the later Abs_reciprocal_sqrt off the critical path
            act_table_hint(AF.Abs_reciprocal_sqrt, f"rs{ln_idx}")
        psum_s = ps.tile([1, NW], F32, tag="bank", bufs=8,
                         name=f"ls_{ln_idx}_{ncq}")
        psum_q = ps.tile([1, NW], F32, tag="bank", bufs=8,
                         name=f"lq_{ln_idx}_{ncq}")
        for do in range(KC):
            dos = slice(do * P, (do + 1) * P)
            pr = bank(f"prj_{pref}_{do}_{ncq}")
            for c in range(okc):
                nc.tensor.matmul(pr, wo_t[:, c, dos], o_tile[:, c, ncs],
                                 start=(c == 0), stop=(c == okc - 1))
            if do > 0:
                _ln_stats(res_out, psum_s, psum_q, ln_idx, ncq, ncs, do - 1,
                          is_f32=False)
            nc.vector.scalar_tensor_tensor(
                out=res_out[:, do, ncs], in0=pr, scalar=bo_t[:, do:do + 1],
                in1=res_in[:, do, ncs], op0=OP.add, op1=OP.add)
        _ln_stats(res_out, psum_s, psum_q, ln_idx, ncq, ncs, KC - 1,
                  is_f32=False)
        nc.vector.tensor_copy(s_row, psum_s)
        nc.vector.tensor_copy(q_row, psum_q)

    def _ln_rows(ln_idx):
        s_row = sb.tile([1, NW], F32, tag="lrow", bufs=6,
                        name=f"srow_{ln_idx}")
        q_row = sb.tile([1, NW], F32, tag="lrow", bufs=6,
                        name=f"qrow_{ln_idx}")
        return s_row, q_row

    def _ln_rowmath(ln_idx, s_row, q_row, eng=None):
        """mu/E[x2] rows -> rstd (fused rsqrt) and mu*rstd, both f32r.
        eng picks the elementwise engine (DVE, or GpSimd to run the chain
        in parallel with DVE work)."""
        eng = eng or nc.vector
        var = sb.tile([1, NW], F32, tag="lrow", bufs=6,
                      name=f"va_{ln_idx}")
        eng.tensor_tensor(var, s_row, s_row, OP.mult)
        eng.tensor_tensor(var, q_row, var, OP.subtract)
        rstd_r = sb.tile([1, NW], F32R, tag="lrow", bufs=6,
                         name=f"rr_{ln_idx}")
        nc.scalar.activation(rstd_r, var, AF.Abs_reciprocal_sqrt,
                             bias=epst[0:1, :])
        murstd = sb.tile([1, NW], F32R, tag="lrow", bufs=6,
                         name=f"ms_{ln_idx}")
        eng.tensor_tensor(murstd, s_row, rstd_r.bitcast(F32), OP.mult)
        return rstd_r, murstd

    def _ln_apply(rT, ln_idx, ncq, rstd_r, murstd, is_f32=False, split=False):
        """PE broadcast of the rows -> in-place normalize of one chunk.
        split=True stages the broadcast rows to SBUF via ACT copies and
        farms two chunks out to GpSimd (which cannot read PSUM) so the
        first chunks of the normalized output land ~2x sooner."""
        ncs = slice(ncq * NW, (ncq + 1) * NW)
        pb_r = ps.tile([P, NW], F32, tag="bank", bufs=8,
                       name=f"pbr_{ln_idx}_{ncq}")
        nc.tensor.matmul(pb_r, onesrow, rstd_r, start=True, stop=True)
        pb_m = ps.tile([P, NW], F32, tag="bank", bufs=8,
                       name=f"pbm_{ln_idx}_{ncq}")
        nc.tensor.matmul(pb_m, onesrow, murstd, start=True, stop=True)
        split = split and not apply_gb
        if split:
            sb_r = sb.tile([P, NW], F32, tag="bct", bufs=2,
                           name=f"sbr_{ln_idx}_{ncq}")
            nc.scalar.activation(sb_r, pb_r, AF.Copy)
            sb_m = sb.tile([P, NW], F32, tag="bct", bufs=2,
                           name=f"sbm_{ln_idx}_{ncq}")
            nc.scalar.activation(sb_m, pb_m, AF.Copy)
        for c in range(KC):
            on_g = split and c in (3, 4)
            e = nc.gpsimd if on_g else nc.vector
            br, bm = (sb_r, sb_m) if on_g else (pb_r, pb_m)
            src = rT[:, c, ncs]
            srcv = src.bitcast(F32) if is_f32 else src
            t1 = sb.tile([P, NW], F32, tag="t1_b", bufs=2,
                         name=f"t1_{ln_idx}_{ncq}_{c}")
            e.tensor_tensor(t1, srcv, br, OP.mult)
            if apply_gb:
                t2 = sb.tile([P, NW], F32, tag="t2_b", bufs=2,
                             name=f"t2_{ln_idx}_{ncq}_{c}")
                nc.vector.tensor_tensor(t2, t1, pb_m, OP.subtract)
                nc.vector.tensor_scalar(
                    out=src, in0=t2,
                    scalar1=lngb[(ln_idx, "g")][:, c:c + 1],
                    scalar2=lngb[(ln_idx, "b")][:, c:c + 1],
                    op0=OP.mult, op1=OP.add)
            else:
                e.tensor_tensor(src, t1, bm, OP.subtract)

    # ---------------- SA out-proj + LN1, per token chunk ---------
    # chunk-0 rowmath (DVE) hides under the chunk-1 projection; chunk-1
    # rowmath runs on GpSimd in parallel with the chunk-0 apply, with the
    # first CA Q-projection halves keeping the PE fed in between.
    r1T = resid_tile("r1T")
    s0, q0 = _ln_rows("1_0")
    out_proj_ln(0, w_sa_o, o_sa, b_sa_bo, xTb, r1T, 1, "r1", s0, q0)
    rm10 = _ln_rowmath("1_0", s0, q0, eng=nc.vector)
    s1, q1 = _ln_rows("1_1")
    out_proj_ln(1, w_sa_o, o_sa, b_sa_bo, xTb, r1T, 1, "r1", s1, q1)
    rm11 = _ln_rowmath("1_1", s1, q1, eng=nc.gpsimd)
    _ln_apply(r1T, 1, 0, *rm10, split=True)
    x1T = r1T

    # ---------------- CA: all Q projections upfront, then attention ------
    # the Q projections (17us of dense PE work) run while LN1 finishes;
    # the attention loop is then a tight software pipeline: scores(h+1)
    # fill the PE while exp(h) runs, normalize(h-1) fills while zrow(h)
    # completes.
    o_ca = sb.tile([P, NH, NT], BF16, tag="opool8", bufs=1, name="o_ca")
    nc.gpsimd.memset(o_ca[96:128, :, :], 0.0)
    qtca = sb.tile([DH, NH, NT], BF16, tag="qkca", bufs=1, name="qtca")

    def ca_q_half(h, ncq):
        hs = slice(h * DH, (h + 1) * DH)
        ncs = slice(ncq * NW, (ncq + 1) * NW)
        pq = ps.tile([DH, NW], F32, tag="bank", bufs=8,
                     name=f"pqca_{h}_{ncq}")
        for c in range(KC):
            nc.tensor.matmul(pq, w_ca_q[:, c, hs], x1T[:, c, ncs],
                             start=(c == 0), stop=(c == KC - 1))
        # alternate copy engines so neither ACT nor DVE serializes the
        # 16 qt copies feeding the attention loop
        if h % 2:
            nc.scalar.activation(qtca[:, h, ncs], pq, AF.Copy)
        else:
            nc.vector.tensor_copy(qtca[:, h, ncs], pq)

    ca_q_half(0, 0)
    ca_q_half(1, 0)
    _ln_apply(r1T, 1, 1, *rm11)
    for h in range(2, NH):
        ca_q_half(h, 0)
    for h in range(NH):
        ca_q_half(h, 1)
    # Exp hint must come AFTER the qtca ACT copies (any ACTIVATE swaps
    # the table) and before the CA scores
    act_table_hint(AF.Exp, "ca")

    def ca_attn(h):
        return attn_scores_av(
            h, qtca[:, h, :],
            kt=lambda mc, _h=h: kt_ca[:, _h, :],
            vaug_sl=lambda mc, _h=h: vca_aug[:, _h, :],
            mchunks=1, mpart=CM, pref="c")

    states = [ca_attn(0), ca_attn(1)]
    states[0][2]()  # finish(0): AV + zrow
    for h in range(1, NH):
        if h + 1 < NH:
            states.append(ca_attn(h + 1))
        attn_normalize(h - 1, states[h - 1][0], states[h - 1][1],
                       o_ca, "c", SPLITS128)
        states[h][2]()
    attn_normalize(NH - 1, states[NH - 1][0], states[NH - 1][1],
                   o_ca, "c", SPLITS128)

    # ---------------- CA out-proj + LN2, per token chunk ---------
    r2T = resid_tile("r2T")
    s20, q20 = _ln_rows("2_0")
    out_proj_ln(0, w_ca_o, o_ca, b_ca_bo, x1T, r2T, 2, "r2", s20, q20,
                okc=NH)
    rm20 = _ln_rowmath("2_0", s20, q20, eng=nc.vector)
    s21, q21 = _ln_rows("2_1")
    out_proj_ln(1, w_ca_o, o_ca, b_ca_bo, x1T, r2T, 2, "r2", s21, q21,
                okc=NH)
    rm21 = _ln_rowmath("2_1", s21, q21, eng=nc.gpsimd)
    _ln_apply(r2T, 2, 0, *rm20, split=True)
    x2T = r2T

    # ---------------- FF (GEGLU): stream w1/w2, FF2 two j behind FF1 ------
    # the lag-2 FF2 keeps the PE queue free of head-of-line blocking when
    # chunk 1's FF2 waits for chunk 0's PSUM banks (released by the
    # chunk-0 stats epilogue, which is injected into chunk 1's j-loop so
    # it runs under PE work instead of in a PE hole at the boundary).
    r3T = resid_tile("r3T", dt=BF16, tag="resid3", bufs=1)
    FFL = 2

    def ff_chunk(ncq, injects):
        """j-streamed GEGLU + interleaved FF2 accumulation for one
        512-token chunk; returns a closure emitting the residual + LN3
        stats epilogue. injects maps j -> callback emitted at that j."""
        ncs = slice(ncq * NW, (ncq + 1) * NW)
        prs = [ps.tile([P, NW], F32, tag="bank", bufs=8,
                       name=f"pr3_{do}_{ncq}") for do in range(KC)]
        mjs = []

        def ff2_mms(j):
            for do in range(KC):
                nc.tensor.matmul(prs[do],
                                 mjs[j][1][:, do * P:(do + 1) * P],
                                 mjs[j][0], start=(j == 0), stop=(j == FJ - 1),
                                 skip_group_check=True)

        for j in range(FJ):
            w1j = sb.tile([P, 2, KC, P], BF16, tag="wff1", bufs=3,
                          name=f"w1j_{ncq}_{j}")
            nc.gpsimd.dma_start(w1j, d["ff_w1_s"][j])
            w2j = sb.tile([P, D], BF16, tag="wff2", bufs=2,
                          name=f"w2j_{ncq}_{j}")
            nc.scalar.dma_start(w2j, d["ff_w2_p"][j])
            pa = ps.tile([P, NW], F32, tag="bank", bufs=8,
                         name=f"pa_{ncq}_{j}")
            pg = ps.tile([P, NW], F32, tag="bank", bufs=8,
                         name=f"pg_{ncq}_{j}")
            for c in range(KC):
                nc.tensor.matmul(pa, w1j[:, 0, c, :], x2T[:, c, ncs],
                                 start=(c == 0), stop=(c == KC - 1))
            for c in range(KC):
                nc.tensor.matmul(pg, w1j[:, 1, c, :], x2T[:, c, ncs],
                                 start=(c == 0), stop=(c == KC - 1))
            if j >= FFL:
                ff2_mms(j - FFL)
            cb = injects.get(j)
            if cb is not None:
                cb()
            gj = sb.tile([P, NW], BF16, tag="gelu", bufs=2,
                         name=f"gj_{ncq}_{j}")
            nc.scalar.activation(gj, pg, AF.Gelu, bias=b_f1g[:, j:j + 1])
            mj = sb.tile([P, NW], BF16, tag="mj", bufs=3,
                         name=f"mj_{ncq}_{j}")
            nc.vector.scalar_tensor_tensor(
                out=mj, in0=pa, scalar=b_f1a[:, j:j + 1],
                in1=gj, op0=OP.add, op1=OP.mult)
            mjs.append((mj, w2j))
        for jj in range(FJ - FFL, FJ - 1):
            ff2_mms(jj)

        def stats_ep():
            # the final FF2 j is emitted per-do here, fused with the
            # residual adds, so each do's stats chain starts as soon as
            # ITS bank stops instead of after the whole final group.
            psum_s = ps.tile([1, NW], F32, tag="bank", bufs=8,
                             name=f"ls_3_{ncq}")
            psum_q = ps.tile([1, NW], F32, tag="bank", bufs=8,
                             name=f"lq_3_{ncq}")
            s_row, q_row = _ln_rows(f"3_{ncq}")
            j = FJ - 1
            for do in range(KC):
                nc.tensor.matmul(prs[do],
                                 mjs[j][1][:, do * P:(do + 1) * P],
                                 mjs[j][0], start=False, stop=True,
                                 skip_group_check=True)
                nc.vector.scalar_tensor_tensor(
                    out=r3T[:, do, ncs], in0=prs[do],
                    scalar=b_ff2[:, do:do + 1],
                    in1=x2T[:, do, ncs], op0=OP.add, op1=OP.add)
                if do > 0:
                    _ln_stats(r3T, psum_s, psum_q, 3, ncq, ncs, do - 1,
                              is_f32=False)
            _ln_stats(r3T, psum_s, psum_q, 3, ncq, ncs, KC - 1, is_f32=False)
            act_table_hint(AF.Abs_reciprocal_sqrt, f"rs3_{ncq}")
            nc.vector.tensor_copy(s_row, psum_s)
            nc.vector.tensor_copy(q_row, psum_q)
            return s_row, q_row
        return stats_ep

    def ff_finalize(ncq, s_row, q_row, eng=None, tail=False):
        ncs = slice(ncq * NW, (ncq + 1) * NW)
        rstd, mur = _ln_rowmath(f"3_{ncq}", s_row, q_row, eng=eng)
        if not tail:
            # split across DVE+GpSimd: this apply is injected into the
            # FF chunk-1 loop, where a DVE-only chain backlogs the mj
            # production that feeds FF2
            _ln_apply(r3T, f"3_{ncq}", ncq, rstd, mur, split=True)
            for c in range(KC):
                nc.sync.dma_start(
                    d["outT"].rearrange("(c p) n -> p c n", p=P)[:, c, ncs],
                    r3T[:, c, ncs])
            return
        # tail: per-chunk apply -> immediate DMA, alternating engines and
        # DMA queues so the last chunk drains as early as possible.
        # GpSimd cannot read PSUM, so the broadcast rows are staged to
        # SBUF first (two cheap DVE copies).
        pb_r = ps.tile([P, NW], F32, tag="bank", bufs=8, name=f"pbr_3t")
        nc.tensor.matmul(pb_r, onesrow, rstd, start=True, stop=True)
        pb_m = ps.tile([P, NW], F32, tag="bank", bufs=8, name=f"pbm_3t")
        nc.tensor.matmul(pb_m, onesrow, mur, start=True, stop=True)
        sb_r = sb.tile([P, NW], F32, tag="bct", bufs=2, name="sbr_3t")
        nc.vector.tensor_copy(sb_r, pb_r)
        sb_m = sb.tile([P, NW], F32, tag="bct", bufs=2, name="sbm_3t")
        nc.vector.tensor_copy(sb_m, pb_m)
        pb_r, pb_m = sb_r, sb_m
        for c in range(KC):
            e = nc.vector if apply_gb else (
                nc.gpsimd if (c % 2) else nc.vector)
            src = r3T[:, c, ncs]
            t1 = sb.tile([P, NW], F32, tag="t1_b", bufs=2,
                         name=f"t1_3t_{c}")
            e.tensor_tensor(t1, src, pb_r, OP.mult)
            if apply_gb:
                t2 = sb.tile([P, NW], F32, tag="t2_b", bufs=2,
                             name=f"t2_3t_{c}")
                e.tensor_tensor(t2, t1, pb_m, OP.subtract)
                e.tensor_scalar(
                    out=src, in0=t2,
                    scalar1=lngb[(3, "g")][:, c:c + 1],
                    scalar2=lngb[(3, "b")][:, c:c + 1],
                    op0=OP.mult, op1=OP.add)
            else:
                e.tensor_tensor(src, t1, pb_m, OP.subtract)
            dq = nc.sync if (c % 2) else nc.scalar
            dq.dma_start(
                d["outT"].rearrange("(c p) n -> p c n", p=P)[:, c, ncs],
                src)

    act_table_hint(AF.Gelu, "ff")  # last ACT op before the first gelu
    ep0 = ff_chunk(0, {1: lambda: _ln_apply(r2T, 2, 1, *rm21, split=True)})
    st0 = {}
    ep1 = ff_chunk(1, {
        1: lambda: st0.update(r=ep0()),
        4: lambda: ff_finalize(0, *st0["r"], eng=nc.gpsimd),
    })
    ff_finalize(1, *ep1(), tail=True)


def _build(apply_gb):
    nc = bacc.Bacc(None, target_bir_lowering=False)
    dt_in = [
        ("xT_bf", [D, NT], BF16),
        ("ctxT_bf", [CD, CM], BF16),
        ("sa_wq_p", [NH, P, KC, DH], BF16), ("sa_wk_p", [NH, P, KC, DH], BF16),
        ("sa_wv_p", [P, KC, D], BF16), ("sa_wo_p", [P, OKC, D], BF16),
        ("ca_wq_p", [P, KC, D], BF16), ("ca_wk_p", [P, CKC, D], BF16),
        ("ca_wv_p", [P, CKC, D], BF16), ("ca_wo_p", [P, NH, D], BF16),
        ("ff_w1_s", [FJ, P, 2, KC, P], BF16), ("ff_w2_p", [FJ, P, D], BF16),
        ("bias_p", [P, 3 * KC + 2 * FJ + 1], F32),
        ("ones", [P, 1], F32R), ("ones_b", [P, 1], BF16),
        ("onesrow", [1, P], F32R),
    ]
    if apply_gb:
        for ln in (1, 2, 3):
            dt_in.append((f"ln{ln}_g_p", [P, KC], F32))
            dt_in.append((f"ln{ln}_b_p", [P, KC], F32))
    nc._kd = {}
    for name, shape, dt in dt_in:
        nc._kd[name] = nc.declare_dram_parameter(name, shape, dt,
                                                 isOutput=False)
    nc._kd["outT"] = nc.declare_dram_parameter("outT", [D, NT], BF16,
                                               isOutput=True)
    with tile.TileContext(nc) as tc:
        _emit(nc, tc, apply_gb)
    nc.compile()
    return nc


def _prep_in_maps(inputs, apply_gb):
    f32 = np.float32
    bf = ml_dtypes.bfloat16
    x = np.asarray(inputs["x"], f32)
    ctx = np.asarray(inputs["context"], f32)

    def pack(w, kc):
        # [kc*128, m] -> [128, kc, m] per-partition contiguous, bf16
        w = np.asarray(w, f32)
        m = w.shape[1]
        return np.ascontiguousarray(
            w.reshape(kc, P, m).transpose(1, 0, 2)).astype(bf)

    def pack_heads(w):
        # [5*128, 8*80] -> [8, 128, 5, 80] head-major so head 0's slab is
        # one small contiguous DMA at startup
        w = np.asarray(w, f32)
        return np.ascontiguousarray(
            w.reshape(KC, P, NH, DH).transpose(2, 1, 0, 3)).astype(bf)

    def pad96(w):
        w = np.asarray(w, f32)
        wp = np.zeros((NH * OPITCH, w.shape[1]), f32)
        for h in range(NH):
            wp[h * OPITCH:h * OPITCH + DH] = w[h * DH:(h + 1) * DH]
        return wp

    def pad128(w):
        w = np.asarray(w, f32)
        wp = np.zeros((NH * P, w.shape[1]), f32)
        for h in range(NH):
            wp[h * P:h * P + DH] = w[h * DH:(h + 1) * DH]
        return wp

    def part(v, cols):
        return np.ascontiguousarray(np.asarray(v, f32).reshape(cols, P).T)

    w1 = np.asarray(inputs["ff_w1"], f32)
    # [c*128+p, s*2560 + j*128 + m] -> [j, p, s, c, m]
    w1s = np.ascontiguousarray(
        w1.reshape(KC, P, 2, FJ, P).transpose(3, 1, 2, 0, 4)).astype(bf)
    w2 = np.asarray(inputs["ff_w2"], f32)
    w2p = np.ascontiguousarray(w2.reshape(FJ, P, D)).astype(bf)

    shared = {
        "sa_wq_p": pack_heads(inputs["sa_wq"]),
        "sa_wk_p": pack_heads(inputs["sa_wk"]),
        "sa_wv_p": pack(inputs["sa_wv"], KC),
        "sa_wo_p": pack(pad96(inputs["sa_wo"]), OKC),
        "ca_wq_p": pack(inputs["ca_wq"], KC),
        "ca_wk_p": pack(inputs["ca_wk"], CKC),
        "ca_wv_p": pack(inputs["ca_wv"], CKC),
        "ca_wo_p": pack(pad128(inputs["ca_wo"]), NH),
        "ff_w1_s": w1s,
        "ff_w2_p": w2p,
        "bias_p": np.concatenate([
            part(inputs["sa_bo"], KC),
            part(inputs["ca_bo"], KC),
            part(inputs["ff_b2"], KC),
            part(np.asarray(inputs["ff_b1"], f32)[:FH], FJ),
            part(np.asarray(inputs["ff_b1"], f32)[FH:], FJ),
            np.full((P, 1), LN_EPS, f32),
        ], axis=1),
        "ones": np.full((P, 1), 1.0 / D, f32),
        "ones_b": np.full((P, 1), 1.0 / D, f32).astype(bf),
        "onesrow": np.ones((1, P), f32),
    }
    if apply_gb:
        for ln in (1, 2, 3):
            shared[f"ln{ln}_g_p"] = part(inputs[f"ln{ln}_g"], KC)
            shared[f"ln{ln}_b_p"] = part(inputs[f"ln{ln}_b"], KC)
    maps = []
    for i in range(B):
        m = dict(shared)
        m["xT_bf"] = np.ascontiguousarray(x[i].T).astype(bf)
        m["ctxT_bf"] = np.ascontiguousarray(ctx[i].T).astype(bf)
        maps.append(m)
    return maps


def _needs_gb(inputs):
    for ln in (1, 2, 3):
        if not np.allclose(np.asarray(inputs[f"ln{ln}_g"]), 1.0):
            return True
        if not np.allclose(np.asarray(inputs[f"ln{ln}_b"]), 0.0):
            return True
    return False


def _run(inputs, trace=False):
    apply_gb = _needs_gb(inputs)
    nc = _build(apply_gb)
    maps = _prep_in_maps(inputs, apply_gb)
    res = run_bass_kernel_spmd(nc, maps, core_ids=list(range(B)), trace=trace)
    out = np.stack([np.asarray(r["outT"]).T for r in res.results])
    return out.astype(np.float32), res


def kernel(**inputs):
    out, _ = _run(inputs, trace=False)
    return out


# revision 88
# speedup vs baseline: 1.0263x; 1.0263x over previous
"""BasicTransformerBlock on 8 TRN2 NeuronCores.

Strategy: pure data parallelism over the batch (B=8, one batch element per
core). Inside each core the block runs with feature-major activations:
  - dense projections: bf16 weights (lhsT) x bf16 activations (rhs)
  - the residual stream is kept in bf16 end-to-end (except the final
    LN3 stage, which stays f32 for the f32 output DMA); rel-err budget
    is 2e-2 and bf16 residual sims at ~8.6e-3
  - attention score/AV in bf16; softmax denominator via a ones-column
    appended to V (extra PSUM partition row), broadcast back across
    partitions with a K=1 ones-matmul on the PE (no DRAM roundtrip)
  - layernorm stats via ones-vector matmuls; mean/rstd rows broadcast
    across partitions with K=1 ones-matmuls
  - attention outputs normalized directly into (head*96+dh)-packed
    128-row tiles (for BOTH attentions) so the output projections
    contract K=128 chunks of the natural wo layout
  - GEGLU FF: ff_w1 streamed per-128-col chunk in bf16, FF2 partial
    products accumulated into 5 dedicated PSUM banks interleaved with
    FF1, so the PE stream stays dense (no mfull staging buffer)
The host pre-transposes x/context, packs all weights into per-partition
contiguous [128, ...] layouts in bf16 (sa q/k head-major so the first
head's slab lands fast at startup), reshapes biases per-partition, and
transposes the output back.
"""

import math

import numpy as np
import ml_dtypes

import concourse.bass as bass
import concourse.mybir as mybir
import concourse.tile as tile
from concourse import bacc
from concourse.bass_utils import run_bass_kernel_spmd

F32 = mybir.dt.float32
F32R = mybir.dt.float32r
BF16 = mybir.dt.bfloat16
AF = mybir.ActivationFunctionType
OP = mybir.AluOpType

P = 128
B = 8
NT = 1024          # query tokens
D = 640            # model dim; 5 chunks of 128
KC = 5
NH = 8             # heads
DH = 80            # head dim
CM = 77            # context tokens
CD = 768           # context dim; 6 chunks
CKC = 6
FH = 2560          # GEGLU half hidden; 20 chunks of 128
FJ = 20
NC = 2             # token chunks of 512
NW = 512
ISCALE = 1.0 / math.sqrt(DH)
LN_EPS = 1e-5

# packed-o layout: head h at rows 96h..96h+80 (96-pitch padding keeps
# every slice's partition base 32-aligned; the zero-padded wo rows make
# the pad partitions contribute nothing to the out-projection).
OPITCH = 96
OKC = NH * OPITCH // P  # 6 chunks of 128 rows
# each head writes its full 96-row pitch: rows 80..96 of po are exact
# zeros (memset pad columns of v_aug), so the pad partitions get zeroed
# as part of the normalize write.  Engines only allow partition-offset
# accesses of <=32 rows (quadrant windows), so any piece whose source or
# destination base is nonzero is cut into 32-row pieces.
O_SPLITS = []
for _h in range(NH):
    _r0, _r1 = _h * OPITCH, (_h + 1) * OPITCH
    _sl = []
    _r = _r0
    while _r < _r1:
        _c = _r // P
        _l = min(_r1, (_c + 1) * P) - _r
        _poff, _rs = _r % P, _r - _r0
        if _poff == 0 and _rs == 0:
            _sl.append((_c, _poff, _rs, _l))
        else:
            for _o in range(0, _l, 32):
                _sl.append((_c, _poff + _o, _rs + _o, min(32, _l - _o)))
        _r += _l
    O_SPLITS.append(_sl)

# CA keeps the simpler 128-pitch o layout: one normalize write per head
# (DVE op cost is column-count driven, so fewer/wider writes beat the
# denser 96-pitch packing there), at the price of 8 instead of 6
# K-chunks in the CA out-projection.
SPLITS128 = [[(_h, 0, 0, OPITCH)] for _h in range(NH)]


def _emit(nc, tc, apply_gb):
    d = nc._kd  # dram handles dict
    with (
        tc.tile_pool(name="sb", bufs=1) as sb,
        tc.tile_pool(name="ps", bufs=1, space="PSUM") as ps,
    ):
        _emit_body(nc, tc, d, sb, ps, apply_gb)


def _emit_body(nc, tc, d, sb, ps, apply_gb):
    def bank(name):
        return ps.tile([P, NW], F32, tag="bank", bufs=8, name=name)

    # ---------------- critical-path loads first: head-0 wq/wk + xTb ----
    # sa q/k weights are head-major [NH, P, KC, DH] in DRAM so head 0's
    # slab (102KB) gates the first matmul instead of the full 819KB.
    w_sa_q = sb.tile([P, NH, KC, DH], BF16, tag="w_sa_q", name="w_sa_q")
    w_sa_k = sb.tile([P, NH, KC, DH], BF16, tag="w_sa_k", name="w_sa_k")
    nc.sync.dma_start(w_sa_q[:, 0], d["sa_wq_p"][0])
    nc.scalar.dma_start(w_sa_k[:, 0], d["sa_wk_p"][0])
    # token-chunk 0 of xTb on the sync queue, chunk 1 on the ACT queue, so
    # the first Q/K projections of both chunks are fed in parallel; V
    # weights go down the gpsimd (SWDGE) queue which is otherwise idle.
    # xTb is the startup DMA bottleneck (~1.3MB, single HWDGE queue runs
    # ~122GB/s): chunks c0-c2 of each token half go down sync/scalar, the
    # c3/c4 chunks down the faster gpsimd SWDGE queue, ahead of wv.
    xTb = sb.tile([P, KC, NT], BF16, tag="xb", bufs=1, name="xTb")
    xsrc = d["xT_bf"].rearrange("(c p) n -> p c n", p=P)
    for ncq in range(NC):
        ncs = slice(ncq * NW, (ncq + 1) * NW)
        eng = nc.sync if ncq == 0 else nc.scalar
        for c in range(3):
            eng.dma_start(xTb[:, c, ncs], xsrc[:, c, ncs])
    for ncq in range(NC):
        ncs = slice(ncq * NW, (ncq + 1) * NW)
        for c in range(3, KC):
            nc.gpsimd.dma_start(xTb[:, c, ncs], xsrc[:, c, ncs])
    w_sa_v = sb.tile([P, KC, D], BF16, tag="w_sa_v", name="w_sa_v")
    nc.gpsimd.dma_start(w_sa_v, d["sa_wv_p"][:, :, :])
    nc.sync.dma_start(w_sa_q[:, 1:],
                      d["sa_wq_p"][1:].rearrange("h p c e -> p h c e"))
    nc.scalar.dma_start(w_sa_k[:, 1:],
                        d["sa_wk_p"][1:].rearrange("h p c e -> p h c e"))

    def w_load(key, shape, name, engine=None):
        t = sb.tile(shape, BF16, tag=name, bufs=1, name=name)
        (engine or nc.gpsimd).dma_start(t, d[key][:, :, :])
        return t

    # ---------------- constant / bias tiles ----------------
    ctxT = sb.tile([P, CKC, CM], BF16, tag="ctxT", bufs=1, name="ctxT")
    nc.gpsimd.dma_start(ctxT, d["ctxT_bf"].rearrange("(c p) m -> p c m", p=P))
    w_ca_k = w_load("ca_wk_p", [P, CKC, D], "w_ca_k")
    w_ca_v = w_load("ca_wv_p", [P, CKC, D], "w_ca_v")
    # all [P, x] fp32 constants in one packed DMA (SWDGE fixed cost ~2us
    # per dma_start; seven tiny loads would stall the early queue)
    NBC = 3 * KC + 2 * FJ + 1
    bias_t = sb.tile([P, NBC], F32, tag="bias", name="bias_t")
    nc.gpsimd.dma_start(bias_t, d["bias_p"][:, :])
    b_sa_bo = bias_t[:, 0:KC]
    b_ca_bo = bias_t[:, KC:2 * KC]
    b_ff2 = bias_t[:, 2 * KC:3 * KC]
    b_f1a = bias_t[:, 3 * KC:3 * KC + FJ]
    b_f1g = bias_t[:, 3 * KC + FJ:3 * KC + 2 * FJ]
    epst = bias_t[:, NBC - 1:NBC]
    ones_r = sb.tile([P, 1], F32R, tag="ones", name="ones_r")
    nc.gpsimd.dma_start(ones_r, d["ones"][:, :])
    ones_b = sb.tile([P, 1], BF16, tag="ones_b", name="ones_b")
    nc.gpsimd.dma_start(ones_b, d["ones_b"][:, :])
    onesrow = sb.tile([1, P], F32R, tag="onesrow", name="onesrow")
    nc.gpsimd.dma_start(onesrow, d["onesrow"][:, :])

    def act_table_hint(func, tag, src=None):
        """tiny dummy activation: forces the ACT table load for `func` to
        happen here (ACT idle) instead of on the critical path."""
        t = sb.tile([1, 8], F32, tag="dummy", bufs=2, name=f"dm_{tag}")
        nc.scalar.activation(t, bias_t[0:1, 0:8] if src is None else src,
                             func)
    lngb = {}
    if apply_gb:
        for ln in (1, 2, 3):
            for gb in ("g", "b"):
                t = sb.tile([P, KC], F32, tag=f"ln{ln}{gb}", name=f"ln{ln}{gb}")
                nc.gpsimd.dma_start(t, d[f"ln{ln}_{gb}_p"][:, :])
                lngb[(ln, gb)] = t

    w_sa_o = w_load("sa_wo_p", [P, OKC, D], "w_sa_o")
    w_ca_q = w_load("ca_wq_p", [P, KC, D], "w_ca_q")
    w_ca_o = w_load("ca_wo_p", [P, NH, D], "w_ca_o")

    def resid_tile(name, dt=BF16, tag="residb", bufs=2):
        return sb.tile([P, KC, NT], dt, tag=tag, bufs=bufs, name=name)

    def qk_proj(h, w_q, w_k, src, pref):
        qt = sb.tile([DH, NT], BF16, tag="qk", bufs=2, name=f"qt{pref}_{h}")
        kt = sb.tile([DH, NT], BF16, tag="qk", bufs=2, name=f"kt{pref}_{h}")
        for ncq in range(NC):
            ncs = slice(ncq * NW, (ncq + 1) * NW)
            pq = ps.tile([DH, NW], F32, tag="bank", bufs=8,
                         name=f"pq{pref}_{h}_{ncq}")
            pk = ps.tile([DH, NW], F32, tag="bank", bufs=8,
                         name=f"pk{pref}_{h}_{ncq}")
            for c in range(KC):
                nc.tensor.matmul(pq, w_q[:, h, c, :], src[:, c, ncs],
                                 start=(c == 0), stop=(c == KC - 1))
            for c in range(KC):
                nc.tensor.matmul(pk, w_k[:, h, c, :], src[:, c, ncs],
                                 start=(c == 0), stop=(c == KC - 1))
            nc.vector.tensor_copy(qt[:, ncs], pq)
            nc.vector.tensor_copy(kt[:, ncs], pk)
        return qt, kt

    # head-0 Q/K first: fills the PE while V weights land
    qk0 = qk_proj(0, w_sa_q, w_sa_k, xTb, "s")

    # ---------------- SA: V projection into V_aug ----------------
    v_aug = sb.tile([P, NH, NH, 97], BF16, tag="vaug", bufs=1, name="v_aug")
    nc.vector.memset(v_aug[:, :, :, 80:96], 0.0)
    nc.vector.memset(v_aug[:, :, :, 96:97], 1.0)
    for tch in range(NH):
        for g in range(2):  # two groups of 4 head-columns (320 wide)
            pv = ps.tile([P, 320], F32, tag="bank", bufs=8, name=f"pv_{tch}_{g}")
            for c in range(KC):
                nc.tensor.matmul(
                    pv, xTb[:, c, tch * P:(tch + 1) * P],
                    w_sa_v[:, c, g * 320:(g + 1) * 320],
                    start=(c == 0), stop=(c == KC - 1))
            nc.vector.tensor_copy(
                v_aug[:, tch, 4 * g:4 * g + 4, 0:80],
                pv.rearrange("p (s e) -> p s e", e=80))

    # preload the Exp table while ACT is still idle (sourced from xTb,
    # which lands ~10us before bias_t on the loaded gpsimd queue)
    act_table_hint(AF.Exp, "sa", src=xTb[0:1, 0, 0:8])

    # ---------------- CA: K/V projections (only need ctx) ----------------
    # emitted from inside the SA loop (h==1) so the PE doesn't wait on the
    # ca weight DMAs at kernel start
    kt_ca = sb.tile([DH, NH, CM], BF16, tag="ktca", bufs=1, name="kt_ca")
    vca_aug = sb.tile([CM, NH, 97], BF16, tag="vca", bufs=1, name="vca_aug")

    def emit_ca_kv():
        for h in range(NH):
            hs = slice(h * DH, (h + 1) * DH)
            pk = ps.tile([DH, CM], F32, tag="bank", bufs=8, name=f"pkca_{h}")
            for c in range(CKC):
                nc.tensor.matmul(pk, w_ca_k[:, c, hs], ctxT[:, c, :],
                                 start=(c == 0), stop=(c == CKC - 1))
            nc.vector.tensor_copy(kt_ca[:, h, :], pk)
        nc.vector.memset(vca_aug[:, :, 80:96], 0.0)
        nc.vector.memset(vca_aug[:, :, 96:97], 1.0)
        for g in range(2):
            pv = ps.tile([CM, 320], F32, tag="bank", bufs=8, name=f"pvca_{g}")
            for c in range(CKC):
                nc.tensor.matmul(pv, ctxT[:, c, :],
                                 w_ca_v[:, c, g * 320:(g + 1) * 320],
                                 start=(c == 0), stop=(c == CKC - 1))
            nc.vector.tensor_copy(vca_aug[:, 4 * g:4 * g + 4, 0:80],
                                  pv.rearrange("p (s e) -> p s e", e=80))

    AVLAG = 2

    def attn_scores_av(h, qt, kt, vaug_sl, mchunks, mpart, pref,
                       zrow_eng=None):
        """scores -> exp -> AV (lagged) for one head; the last AVLAG AV
        matmuls are deferred to the returned closure so the caller can
        slot independent PE work (next head's projections) in front of
        them while the exps complete.  zrow_eng picks the engine for the
        denominator-row copy: DVE in the ACT-bound SA phase, ACT in the
        DVE-leaning CA phase."""
        pos, tails = [], []
        for ncq in range(NC):
            ncs = slice(ncq * NW, (ncq + 1) * NW)
            po = ps.tile([97, NW], F32, tag="bank", bufs=8,
                         name=f"po{pref}_{h}_{ncq}")
            ets = []
            for mc in range(mchunks):
                pscore = ps.tile([mpart, NW], F32, tag="bank", bufs=8,
                                 name=f"psc{pref}_{h}_{ncq}_{mc}")
                nc.tensor.matmul(pscore, kt(mc), qt[:, ncs],
                                 start=True, stop=True)
                et = sb.tile([mpart, NW], BF16, tag="epool", bufs=4,
                             name=f"e{pref}_{h}_{ncq}_{mc}")
                nc.scalar.activation(et, pscore, AF.Exp, scale=ISCALE)
                ets.append(et)
                if mc >= AVLAG:  # AV lags its exp by AVLAG scores
                    nc.tensor.matmul(po, vaug_sl(mc - AVLAG), ets[mc - AVLAG],
                                     start=(mc == AVLAG), stop=False,
                                     skip_group_check=True)
            pos.append(po)
            tails.append(ets)

        zrows = []

        def finish():
            for ncq in range(NC):
                po, ets = pos[ncq], tails[ncq]
                for mc in range(max(0, mchunks - AVLAG), mchunks):
                    nc.tensor.matmul(po, vaug_sl(mc), ets[mc],
                                     start=(mc == 0),
                                     stop=(mc == mchunks - 1),
                                     skip_group_check=True)
                zrow = sb.tile([1, NW], F32R, tag="zrow", bufs=1,
                               name=f"zr{pref}_{h}_{ncq}")
                if zrow_eng is nc.vector:
                    nc.vector.tensor_copy(zrow, po[96:97, :])
                else:
                    nc.scalar.activation(zrow, po[96:97, :], AF.Copy)
                zrows.append(zrow)
        return pos, zrows, finish

    def attn_normalize(h, pos, zrows, o_tile, pref, splits):
        """softmax denominator -> PE broadcast -> recip -> packed write."""
        for ncq in range(NC):
            ncs = slice(ncq * NW, (ncq + 1) * NW)
            po = pos[ncq]
            pzb = ps.tile([OPITCH, NW], F32, tag="bank", bufs=8,
                          name=f"pzb{pref}_{h}_{ncq}")
            nc.tensor.matmul(pzb, onesrow[0:1, 0:OPITCH], zrows[ncq],
                             start=True, stop=True)
            zb = sb.tile([OPITCH, NW], F32, tag="zb", bufs=1,
                         name=f"zb{pref}_{h}_{ncq}")
            nc.vector.reciprocal_approx_fast(zb, pzb)
            for (c, poff, rs, ln) in splits[h]:
                nc.vector.tensor_tensor(
                    o_tile[poff:poff + ln, c, ncs],
                    po[rs:rs + ln, :], zb[rs:rs + ln, :], OP.mult)

    # ---------------- SA: per-head QK + attention ----------------
    # next head's Q/K projections are emitted between this head's AV and
    # its normalize so the PE never idles on the denominator chain.
    o_sa = sb.tile([P, OKC, NT], BF16, tag="opool", bufs=1, name="o_sa")
    qkh = qk0
    for h in range(NH):
        qt, kt = qkh
        pos, zrows, finish = attn_scores_av(
            h, qt,
            kt=lambda mc, _kt=kt: _kt[:, mc * P:(mc + 1) * P],
            vaug_sl=lambda mc, _h=h: v_aug[:, mc, _h, :],
            mchunks=NH, mpart=P, pref="s", zrow_eng=nc.vector)
        if h + 1 < NH:
            qkh = qk_proj(h + 1, w_sa_q, w_sa_k, xTb, "s")
        if h == 1:
            emit_ca_kv()
        finish()
        attn_normalize(h, pos, zrows, o_sa, "s", O_SPLITS)

    def _ln_stats(res_out, psum_s, psum_q, ln_idx, ncq, ncs, do, is_f32):
        """sum and sum-of-squares matmuls for one 128-feature chunk."""
        src = res_out[:, do, ncs]
        sq = sb.tile([P, NW], F32R, tag="sq", bufs=1,
                     name=f"sq_{ln_idx}_{ncq}_{do}")
        nc.scalar.activation(sq, src.bitcast(F32) if is_f32 else src,
                             AF.Square)
        nc.tensor.matmul(psum_s, ones_r if is_f32 else ones_b, src,
                         start=(do == 0), stop=(do == KC - 1))
        nc.tensor.matmul(psum_q, ones_r, sq,
                         start=(do == 0), stop=(do == KC - 1))

    def out_proj_ln(ncq, wo_t, o_tile, bo_t, res_in, res_out, ln_idx, pref,
                    s_row, q_row, okc=OKC):
        """packed out-projection + residual + LN stats for one 512-token
        chunk; stats matmuls pipelined one chunk behind the residual."""
        ncs = slice(ncq * NW, (ncq + 1) * NW)
        if ncq == 0:
            # squares also exist in the rsqrt table; switching now keeps
            # the later Abs_reciprocal_sqrt off the critical path
            act_table_hint(AF.Abs_reciprocal_sqrt, f"rs{ln_idx}")
        psum_s = ps.tile([1, NW], F32, tag="bank", bufs=8,
                         name=f"ls_{ln_idx}_{ncq}")
        psum_q = ps.tile([1, NW], F32, tag="bank", bufs=8,
                         name=f"lq_{ln_idx}_{ncq}")
        for do in range(KC):
            dos = slice(do * P, (do + 1) * P)
            pr = bank(f"prj_{pref}_{do}_{ncq}")
            for c in range(okc):
                nc.tensor.matmul(pr, wo_t[:, c, dos], o_tile[:, c, ncs],
                                 start=(c == 0), stop=(c == okc - 1))
            if do > 0:
                _ln_stats(res_out, psum_s, psum_q, ln_idx, ncq, ncs, do - 1,
                          is_f32=False)
            nc.vector.scalar_tensor_tensor(
                out=res_out[:, do, ncs], in0=pr, scalar=bo_t[:, do:do + 1],
                in1=res_in[:, do, ncs], op0=OP.add, op1=OP.add)
        _ln_stats(res_out, psum_s, psum_q, ln_idx, ncq, ncs, KC - 1,
                  is_f32=False)
        nc.vector.tensor_copy(s_row, psum_s)
        nc.vector.tensor_copy(q_row, psum_q)

    def _ln_rows(ln_idx):
        s_row = sb.tile([1, NW], F32, tag="lrow", bufs=6,
                        name=f"srow_{ln_idx}")
        q_row = sb.tile([1, NW], F32, tag="lrow", bufs=6,
                        name=f"qrow_{ln_idx}")
        return s_row, q_row

    def _ln_rowmath(ln_idx, s_row, q_row, eng=None):
        """mu/E[x2] rows -> rstd (fused rsqrt) and mu*rstd, both f32r.
        eng picks the elementwise engine (DVE, or GpSimd to run the chain
        in parallel with DVE work)."""
        eng = eng or nc.vector
        var = sb.tile([1, NW], F32, tag="lrow", bufs=6,
                      name=f"va_{ln_idx}")
        eng.tensor_tensor(var, s_row, s_row, OP.mult)
        eng.tensor_tensor(var, q_row, var, OP.subtract)
        rstd_r = sb.tile([1, NW], F32R, tag="lrow", bufs=6,
                         name=f"rr_{ln_idx}")
        nc.scalar.activation(rstd_r, var, AF.Abs_reciprocal_sqrt,
                             bias=epst[0:1, :])
        murstd = sb.tile([1, NW], F32R, tag="lrow", bufs=6,
                         name=f"ms_{ln_idx}")
        eng.tensor_tensor(murstd, s_row, rstd_r.bitcast(F32), OP.mult)
        return rstd_r, murstd

    def _ln_apply(rT, ln_idx, ncq, rstd_r, murstd, is_f32=False, split=False):
        """PE broadcast of the rows -> in-place normalize of one chunk.
        split=True stages the broadcast rows to SBUF via ACT copies and
        farms two chunks out to GpSimd (which cannot read PSUM) so the
        first chunks of the normalized output land ~2x sooner."""
        ncs = slice(ncq * NW, (ncq + 1) * NW)
        pb_r = ps.tile([P, NW], F32, tag="bank", bufs=8,
                       name=f"pbr_{ln_idx}_{ncq}")
        nc.tensor.matmul(pb_r, onesrow, rstd_r, start=True, stop=True)
        pb_m = ps.tile([P, NW], F32, tag="bank", bufs=8,
                       name=f"pbm_{ln_idx}_{ncq}")
        nc.tensor.matmul(pb_m, onesrow, murstd, start=True, stop=True)
        split = split and not apply_gb
        if split:
            sb_r = sb.tile([P, NW], F32, tag="bct", bufs=2,
                           name=f"sbr_{ln_idx}_{ncq}")
            nc.scalar.activation(sb_r, pb_r, AF.Copy)
            sb_m = sb.tile([P, NW], F32, tag="bct", bufs=2,
                           name=f"sbm_{ln_idx}_{ncq}")
            nc.scalar.activation(sb_m, pb_m, AF.Copy)
        for c in range(KC):
            on_g = split and c in (3, 4)
            e = nc.gpsimd if on_g else nc.vector
            br, bm = (sb_r, sb_m) if on_g else (pb_r, pb_m)
            src = rT[:, c, ncs]
            srcv = src.bitcast(F32) if is_f32 else src
            t1 = sb.tile([P, NW], F32, tag="t1_b", bufs=2,
                         name=f"t1_{ln_idx}_{ncq}_{c}")
            e.tensor_tensor(t1, srcv, br, OP.mult)
            if apply_gb:
                t2 = sb.tile([P, NW], F32, tag="t2_b", bufs=2,
                             name=f"t2_{ln_idx}_{ncq}_{c}")
                nc.vector.tensor_tensor(t2, t1, pb_m, OP.subtract)
                nc.vector.tensor_scalar(
                    out=src, in0=t2,
                    scalar1=lngb[(ln_idx, "g")][:, c:c + 1],
                    scalar2=lngb[(ln_idx, "b")][:, c:c + 1],
                    op0=OP.mult, op1=OP.add)
            else:
                e.tensor_tensor(src, t1, bm, OP.subtract)

    # ---------------- SA out-proj + LN1, per token chunk ---------
    # chunk-0 rowmath (DVE) hides under the chunk-1 projection; chunk-1
    # rowmath runs on GpSimd in parallel with the chunk-0 apply, with the
    # first CA Q-projection halves keeping the PE fed in between.
    r1T = resid_tile("r1T")
    s0, q0 = _ln_rows("1_0")
    out_proj_ln(0, w_sa_o, o_sa, b_sa_bo, xTb, r1T, 1, "r1", s0, q0)
    rm10 = _ln_rowmath("1_0", s0, q0, eng=nc.vector)
    s1, q1 = _ln_rows("1_1")
    out_proj_ln(1, w_sa_o, o_sa, b_sa_bo, xTb, r1T, 1, "r1", s1, q1)
    rm11 = _ln_rowmath("1_1", s1, q1, eng=nc.gpsimd)
    _ln_apply(r1T, 1, 0, *rm10, split=True)
    x1T = r1T

    # ---------------- CA: all Q projections upfront, then attention ------
    # the Q projections (17us of dense PE work) run while LN1 finishes;
    # the attention loop is then a tight software pipeline: scores(h+1)
    # fill the PE while exp(h) runs, normalize(h-1) fills while zrow(h)
    # completes.
    o_ca = sb.tile([P, NH, NT], BF16, tag="opool8", bufs=1, name="o_ca")
    nc.gpsimd.memset(o_ca[96:128, :, :], 0.0)
    qtca = sb.tile([DH, NH, NT], BF16, tag="qkca", bufs=1, name="qtca")

    def ca_q_half(h, ncq):
        hs = slice(h * DH, (h + 1) * DH)
        ncs = slice(ncq * NW, (ncq + 1) * NW)
        pq = ps.tile([DH, NW], F32, tag="bank", bufs=8,
                     name=f"pqca_{h}_{ncq}")
        for c in range(KC):
            nc.tensor.matmul(pq, w_ca_q[:, c, hs], x1T[:, c, ncs],
                             start=(c == 0), stop=(c == KC - 1))
        # alternate copy engines so neither ACT nor DVE serializes the
        # 16 qt copies feeding the attention loop
        if h % 2:
            nc.scalar.activation(qtca[:, h, ncs], pq, AF.Copy)
        else:
            nc.vector.tensor_copy(qtca[:, h, ncs], pq)

    ca_q_half(0, 0)
    ca_q_half(1, 0)
    _ln_apply(r1T, 1, 1, *rm11)
    for h in range(2, NH):
        ca_q_half(h, 0)
    for h in range(NH):
        ca_q_half(h, 1)
    # Exp hint must come AFTER the qtca ACT copies (any ACTIVATE swaps
    # the table) and before the CA scores
    act_table_hint(AF.Exp, "ca")

    def ca_attn(h):
        return attn_scores_av(
            h, qtca[:, h, :],
            kt=lambda mc, _h=h: kt_ca[:, _h, :],
            vaug_sl=lambda mc, _h=h: vca_aug[:, _h, :],
            mchunks=1, mpart=CM, pref="c")

    states = [ca_attn(0), ca_attn(1)]
    states[0][2]()  # finish(0): AV + zrow
    for h in range(1, NH):
        if h + 1 < NH:
            states.append(ca_attn(h + 1))
        attn_normalize(h - 1, states[h - 1][0], states[h - 1][1],
                       o_ca, "c", SPLITS128)
        states[h][2]()
    attn_normalize(NH - 1, states[NH - 1][0], states[NH - 1][1],
                   o_ca, "c", SPLITS128)

    # ---------------- CA out-proj + LN2, per token chunk ---------
    r2T = resid_tile("r2T")
    s20, q20 = _ln_rows("2_0")
    out_proj_ln(0, w_ca_o, o_ca, b_ca_bo, x1T, r2T, 2, "r2", s20, q20,
                okc=NH)
    rm20 = _ln_rowmath("2_0", s20, q20, eng=nc.vector)
    s21, q21 = _ln_rows("2_1")
    out_proj_ln(1, w_ca_o, o_ca, b_ca_bo, x1T, r2T, 2, "r2", s21, q21,
                okc=NH)
    rm21 = _ln_rowmath("2_1", s21, q21, eng=nc.gpsimd)
    _ln_apply(r2T, 2, 0, *rm20, split=True)
    x2T = r2T

    # ---------------- FF (GEGLU): stream w1/w2, FF2 two j behind FF1 ------
    # the lag-2 FF2 keeps the PE queue free of head-of-line blocking when
    # chunk 1's FF2 waits for chunk 0's PSUM banks (released by the
    # chunk-0 stats epilogue, which is injected into chunk 1's j-loop so
    # it runs under PE work instead of in a PE hole at the boundary).
    r3T = resid_tile("r3T", dt=BF16, tag="resid3", bufs=1)
    FFL = 2

    def ff_chunk(ncq, injects):
        """j-streamed GEGLU + interleaved FF2 accumulation for one
        512-token chunk; returns a closure emitting the residual + LN3
        stats epilogue. injects maps j -> callback emitted at that j."""
        ncs = slice(ncq * NW, (ncq + 1) * NW)
        prs = [ps.tile([P, NW], F32, tag="bank", bufs=8,
                       name=f"pr3_{do}_{ncq}") for do in range(KC)]
        mjs = []

        def ff2_mms(j):
            for do in range(KC):
                nc.tensor.matmul(prs[do],
                                 mjs[j][1][:, do * P:(do + 1) * P],
                                 mjs[j][0], start=(j == 0), stop=(j == FJ - 1),
                                 skip_group_check=True)

        for j in range(FJ):
            w1j = sb.tile([P, 2, KC, P], BF16, tag="wff1", bufs=3,
                          name=f"w1j_{ncq}_{j}")
            nc.gpsimd.dma_start(w1j, d["ff_w1_s"][j])
            w2j = sb.tile([P, D], BF16, tag="wff2", bufs=2,
                          name=f"w2j_{ncq}_{j}")
            nc.scalar.dma_start(w2j, d["ff_w2_p"][j])
            pa = ps.tile([P, NW], F32, tag="bank", bufs=8,
                         name=f"pa_{ncq}_{j}")
            pg = ps.tile([P, NW], F32, tag="bank", bufs=8,
                         name=f"pg_{ncq}_{j}")
            for c in range(KC):
                nc.tensor.matmul(pa, w1j[:, 0, c, :], x2T[:, c, ncs],
                                 start=(c == 0), stop=(c == KC - 1))
            for c in range(KC):
                nc.tensor.matmul(pg, w1j[:, 1, c, :], x2T[:, c, ncs],
                                 start=(c == 0), stop=(c == KC - 1))
            if j >= FFL:
                ff2_mms(j - FFL)
            cb = injects.get(j)
            if cb is not None:
                cb()
            gj = sb.tile([P, NW], BF16, tag="gelu", bufs=2,
                         name=f"gj_{ncq}_{j}")
            nc.scalar.activation(gj, pg, AF.Gelu, bias=b_f1g[:, j:j + 1])
            mj = sb.tile([P, NW], BF16, tag="mj", bufs=3,
                         name=f"mj_{ncq}_{j}")
            nc.vector.scalar_tensor_tensor(
                out=mj, in0=pa, scalar=b_f1a[:, j:j + 1],
                in1=gj, op0=OP.add, op1=OP.mult)
            mjs.append((mj, w2j))
        for jj in range(FJ - FFL, FJ - 1):
            ff2_mms(jj)

        def stats_ep():
            # the final FF2 j is emitted per-do here, fused with the
            # residual adds, so each do's stats chain starts as soon as
            # ITS bank stops instead of after the whole final group.
            psum_s = ps.tile([1, NW], F32, tag="bank", bufs=8,
                             name=f"ls_3_{ncq}")
            psum_q = ps.tile([1, NW], F32, tag="bank", bufs=8,
                             name=f"lq_3_{ncq}")
            s_row, q_row = _ln_rows(f"3_{ncq}")
            j = FJ - 1
            for do in range(KC):
                nc.tensor.matmul(prs[do],
                                 mjs[j][1][:, do * P:(do + 1) * P],
                                 mjs[j][0], start=False, stop=True,
                                 skip_group_check=True)
                nc.vector.scalar_tensor_tensor(
                    out=r3T[:, do, ncs], in0=prs[do],
                    scalar=b_ff2[:, do:do + 1],
                    in1=x2T[:, do, ncs], op0=OP.add, op1=OP.add)
                if do > 0:
                    _ln_stats(r3T, psum_s, psum_q, 3, ncq, ncs, do - 1,
                              is_f32=False)
            _ln_stats(r3T, psum_s, psum_q, 3, ncq, ncs, KC - 1, is_f32=False)
            act_table_hint(AF.Abs_reciprocal_sqrt, f"rs3_{ncq}")
            nc.vector.tensor_copy(s_row, psum_s)
            nc.vector.tensor_copy(q_row, psum_q)
            return s_row, q_row
        return stats_ep

    def ff_finalize(ncq, s_row, q_row, eng=None, tail=False):
        ncs = slice(ncq * NW, (ncq + 1) * NW)
        rstd, mur = _ln_rowmath(f"3_{ncq}", s_row, q_row, eng=eng)
        if not tail:
            # split across DVE+GpSimd: this apply is injected into the
            # FF chunk-1 loop, where a DVE-only chain backlogs the mj
            # production that feeds FF2
            _ln_apply(r3T, f"3_{ncq}", ncq, rstd, mur, split=True)
            for c in range(KC):
                nc.sync.dma_start(
                    d["outT"].rearrange("(c p) n -> p c n", p=P)[:, c, ncs],
                    r3T[:, c, ncs])
            return
        # tail: per-chunk apply -> immediate DMA, alternating engines and
        # DMA queues so the last chunk drains as early as possible.
        # GpSimd cannot read PSUM, so the broadcast rows are staged to
        # SBUF first (two cheap DVE copies).
        pb_r = ps.tile([P, NW], F32, tag="bank", bufs=8, name=f"pbr_3t")
        nc.tensor.matmul(pb_r, onesrow, rstd, start=True, stop=True)
        pb_m = ps.tile([P, NW], F32, tag="bank", bufs=8, name=f"pbm_3t")
        nc.tensor.matmul(pb_m, onesrow, mur, start=True, stop=True)
        sb_r = sb.tile([P, NW], F32, tag="bct", bufs=2, name="sbr_3t")
        nc.vector.tensor_copy(sb_r, pb_r)
        sb_m = sb.tile([P, NW], F32, tag="bct", bufs=2, name="sbm_3t")
        nc.vector.tensor_copy(sb_m, pb_m)
        pb_r, pb_m = sb_r, sb_m
        for c in range(KC):
            e = nc.vector if apply_gb else (
                nc.gpsimd if (c % 2) else nc.vector)
            src = r3T[:, c, ncs]
            t1 = sb.tile([P, NW], F32, tag="t1_b", bufs=2,
                         name=f"t1_3t_{c}")
            e.tensor_tensor(t1, src, pb_r, OP.mult)
            if apply_gb:
                t2 = sb.tile([P, NW], F32, tag="t2_b", bufs=2,
                             name=f"t2_3t_{c}")
                e.tensor_tensor(t2, t1, pb_m, OP.subtract)
                e.tensor_scalar(
                    out=src, in0=t2,
                    scalar1=lngb[(3, "g")][:, c:c + 1],
                    scalar2=lngb[(3, "b")][:, c:c + 1],
                    op0=OP.mult, op1=OP.add)
            else:
                e.tensor_tensor(src, t1, pb_m, OP.subtract)
            dq = nc.sync if (c % 2) else nc.scalar
            dq.dma_start(
                d["outT"].rearrange("(c p) n -> p c n", p=P)[:, c, ncs],
                src)

    act_table_hint(AF.Gelu, "ff")  # last ACT op before the first gelu
    ep0 = ff_chunk(0, {1: lambda: _ln_apply(r2T, 2, 1, *rm21, split=True)})
    st0 = {}
    ep1 = ff_chunk(1, {
        1: lambda: st0.update(r=ep0()),
        4: lambda: ff_finalize(0, *st0["r"], eng=nc.gpsimd),
    })
    ff_finalize(1, *ep1(), tail=True)


def _build(apply_gb):
    nc = bacc.Bacc(None, target_bir_lowering=False)
    dt_in = [
        ("xT_bf", [D, NT], BF16),
        ("ctxT_bf", [CD, CM], BF16),
        ("sa_wq_p", [NH, P, KC, DH], BF16), ("sa_wk_p", [NH, P, KC, DH], BF16),
        ("sa_wv_p", [P, KC, D], BF16), ("sa_wo_p", [P, OKC, D], BF16),
        ("ca_wq_p", [P, KC, D], BF16), ("ca_wk_p", [P, CKC, D], BF16),
        ("ca_wv_p", [P, CKC, D], BF16), ("ca_wo_p", [P, NH, D], BF16),
        ("ff_w1_s", [FJ, P, 2, KC, P], BF16), ("ff_w2_p", [FJ, P, D], BF16),
        ("bias_p", [P, 3 * KC + 2 * FJ + 1], F32),
        ("ones", [P, 1], F32R), ("ones_b", [P, 1], BF16),
        ("onesrow", [1, P], F32R),
    ]
    if apply_gb:
        for ln in (1, 2, 3):
            dt_in.append((f"ln{ln}_g_p", [P, KC], F32))
            dt_in.append((f"ln{ln}_b_p", [P, KC], F32))
    nc._kd = {}
    for name, shape, dt in dt_in:
        nc._kd[name] = nc.declare_dram_parameter(name, shape, dt,
                                                 isOutput=False)
    nc._kd["outT"] = nc.declare_dram_parameter("outT", [D, NT], BF16,
                                               isOutput=True)
    with tile.TileContext(nc) as tc:
        _emit(nc, tc, apply_gb)
    nc.compile()
    return nc


def _prep_in_maps(inputs, apply_gb):
    f32 = np.float32
    bf = ml_dtypes.bfloat16
    x = np.asarray(inputs["x"], f32)
    ctx = np.asarray(inputs["context"], f32)

    def pack(w, kc):
        # [kc*128, m] -> [128, kc, m] per-partition contiguous, bf16
        w = np.asarray(w, f32)
        m = w.shape[1]
        return np.ascontiguousarray(
            w.reshape(kc, P, m).transpose(1, 0, 2)).astype(bf)

    def pack_heads(w):
        # [5*128, 8*80] -> [8, 128, 5, 80] head-major so head 0's slab is
        # one small contiguous DMA at startup
        w = np.asarray(w, f32)
        return np.ascontiguousarray(
            w.reshape(KC, P, NH, DH).transpose(2, 1, 0, 3)).astype(bf)

    def pad96(w):
        w = np.asarray(w, f32)
        wp = np.zeros((NH * OPITCH, w.shape[1]), f32)
        for h in range(NH):
            wp[h * OPITCH:h * OPITCH + DH] = w[h * DH:(h + 1) * DH]
        return wp

    def pad128(w):
        w = np.asarray(w, f32)
        wp = np.zeros((NH * P, w.shape[1]), f32)
        for h in range(NH):
            wp[h * P:h * P + DH] = w[h * DH:(h + 1) * DH]
        return wp

    def part(v, cols):
        return np.ascontiguousarray(np.asarray(v, f32).reshape(cols, P).T)

    w1 = np.asarray(inputs["ff_w1"], f32)
    # [c*128+p, s*2560 + j*128 + m] -> [j, p, s, c, m]
    w1s = np.ascontiguousarray(
        w1.reshape(KC, P, 2, FJ, P).transpose(3, 1, 2, 0, 4)).astype(bf)
    w2 = np.asarray(inputs["ff_w2"], f32)
    w2p = np.ascontiguousarray(w2.reshape(FJ, P, D)).astype(bf)

    shared = {
        "sa_wq_p": pack_heads(inputs["sa_wq"]),
        "sa_wk_p": pack_heads(inputs["sa_wk"]),
        "sa_wv_p": pack(inputs["sa_wv"], KC),
        "sa_wo_p": pack(pad96(inputs["sa_wo"]), OKC),
        "ca_wq_p": pack(inputs["ca_wq"], KC),
        "ca_wk_p": pack(inputs["ca_wk"], CKC),
        "ca_wv_p": pack(inputs["ca_wv"], CKC),
        "ca_wo_p": pack(pad128(inputs["ca_wo"]), NH),
        "ff_w1_s": w1s,
        "ff_w2_p": w2p,
        "bias_p": np.concatenate([
            part(inputs["sa_bo"], KC),
            part(inputs["ca_bo"], KC),
            part(inputs["ff_b2"], KC),
            part(np.asarray(inputs["ff_b1"], f32)[:FH], FJ),
            part(np.asarray(inputs["ff_b1"], f32)[FH:], FJ),
            np.full((P, 1), LN_EPS, f32),
        ], axis=1),
        "ones": np.full((P, 1), 1.0 / D, f32),
        "ones_b": np.full((P, 1), 1.0 / D, f32).astype(bf),
        "onesrow": np.ones((1, P), f32),
    }
    if apply_gb:
        for ln in (1, 2, 3):
            shared[f"ln{ln}_g_p"] = part(inputs[f"ln{ln}_g"], KC)
            shared[f"ln{ln}_b_p"] = part(inputs[f"ln{ln}_b"], KC)
    maps = []
    for i in range(B):
        m = dict(shared)
        m["xT_bf"] = np.ascontiguousarray(x[i].T).astype(bf)
        m["ctxT_bf"] = np.ascontiguousarray(ctx[i].T).astype(bf)
        maps.append(m)
    return maps


def _needs_gb(inputs):
    for ln in (1, 2, 3):
        if not np.allclose(np.asarray(inputs[f"ln{ln}_g"]), 1.0):
            return True
        if not np.allclose(np.asarray(inputs[f"ln{ln}_b"]), 0.0):
            return True
    return False


def _run(inputs, trace=False):
    apply_gb = _needs_gb(inputs)
    nc = _build(apply_gb)
    maps = _prep_in_maps(inputs, apply_gb)
    res = run_bass_kernel_spmd(nc, maps, core_ids=list(range(B)), trace=trace)
    out = np.stack([np.asarray(r["outT"]).T for r in res.results])
    return out.astype(np.float32), res


def kernel(**inputs):
    out, _ = _run(inputs, trace=False)
    return out
